# revision 12
# baseline (speedup 1.0000x reference)
"""Trainium2 Bass kernel for unscaled Luong dot-product attention.

Problem: B=16, Tq=Tk=D=1024, fp32.
    scores = Q @ E^T ; weights = softmax(scores, -1) ; out = weights @ E

Sharding: data-parallel over batch — each of the 8 NeuronCores processes
2 batches end-to-end; no cross-core communication.

Layout strategy: the host-side sharding step (inside kernel()) rearranges
each core's inputs so no on-device transposition of Q or E is needed:
  - q is shipped per q-block as [qb, d-part, dc, j] (i.e. Q^T tiled), so
    each 128-row q-block's stationary operands DMA straight into SBUF.
  - e is shipped twice: natural [k-part, kc, d] (bmm2 rhs) and transposed
    [d-part, dc, k] (bmm1 rhs). One 4 MB DMA each per batch.
q/et are declared float32r: the PE reads the raw fp32 bits at its
full-rate reduced internal precision (~15-16 effective mantissa bits).
The weights/bmm2 path runs in bf16 (softmax weights are smooth in
[0,1]; error there is not amplified by the softmax). Measured rel_l2 =
1.85e-3 vs the fp32 reference; the gate is 2e-2. Only the softmax
weights, which are produced on device, still go through a PE transpose
(bf16, 1 cyc/row).

Per-core pipeline per batch, per 128-row q-block (software-pipelined:
block qb+1's bmm1 overlaps block qb's softmax/bmm2 tail):
  front: DMA q-block tiles, bmm1 into PSUM kh-outer (the row-max of
    half 0 starts at the halfway point).
  back: negated row-max (DVE) -> exp with per-partition bias and fused
    row-sum (ACT, bf16 out) -> PE-transpose W (bf16) -> bmm2 kc-outer
    with paired stationary operands -> fold 1/rowsum into the
    PSUM->SBUF output copy (DVE) -> DMA out.

Measured (paired reps=32 differencing): ~146 us/core steady state, vs
324 us for the 3xTF32-split predecessor. An mm-only ablation times the
same, i.e. the matmul stream (incl. its serial per-MM weight loads) is
the wall; softmax/transposes are fully hidden.
"""

import numpy as np

import concourse.bass as bass
import concourse.tile as tile
from concourse import bacc, mybir
from concourse.masks import make_identity

P = 128
B_PER_CORE = 2
T = 1024  # Tq = Tk
D = 1024
NC_CHUNKS = T // P  # 8 k-chunks / q-blocks
ND_CHUNKS = D // P  # 8 d-chunks
F32 = mybir.dt.float32
F32R = mybir.dt.float32r
BF16 = mybir.dt.bfloat16


def build_nc(reps: int = 1, mm_only: bool = False):
    nc = bacc.Bacc("TRN2", target_bir_lowering=False, debug=False)
    # q: [b, qb, p, dc*128+j] = Q[b, qb*128+j, dc*128+p]  (Q^T, block-tiled)
    q_dram = nc.dram_tensor(
        "q", [B_PER_CORE, NC_CHUNKS, P, D], F32R, kind="ExternalInput"
    ).ap()
    # e: [b, p, kc*1024+d] = E[b, kc*128+p, d]  (natural, partition-tiled,
    # bf16 — bmm2's rhs; the softmax weights are bf16 anyway)
    e_dram = nc.dram_tensor(
        "e", [B_PER_CORE, P, NC_CHUNKS * D], BF16, kind="ExternalInput"
    ).ap()
    # et: [b, p, dc*1024+k] = E[b, k, dc*128+p]  (transposed, partition-tiled)
    et_dram = nc.dram_tensor(
        "et", [B_PER_CORE, P, ND_CHUNKS * T], F32R, kind="ExternalInput"
    ).ap()
    o_dram = nc.dram_tensor("o", [B_PER_CORE, T, D], F32, kind="ExternalOutput").ap()

    with tile.TileContext(nc) as tc:
        with (
            tc.tile_pool(name="const", bufs=1) as const_pool,
            tc.tile_pool(name="e_r", bufs=2) as e_r_pool,
            tc.tile_pool(name="etr", bufs=2) as etr_pool,
            tc.tile_pool(name="qt", bufs=3) as qt_pool,
            tc.tile_pool(name="w", bufs=2) as w_pool,
            tc.tile_pool(name="wt", bufs=2) as wt_pool,
            tc.tile_pool(name="ctx", bufs=2) as ctx_pool,
            tc.tile_pool(name="stat", bufs=4) as stat_pool,
            tc.tile_pool(name="sc_ps", bufs=2, space="PSUM") as sc_psum,
            tc.tile_pool(name="ctx_ps", bufs=1, space="PSUM") as ctx_psum,
            tc.tile_pool(name="tr_ps", bufs=2, space="PSUM") as trans_psum,
        ):
            ident = const_pool.tile([P, P], F32)
            make_identity(nc, ident[:])
            ident_b = const_pool.tile([P, P], BF16)
            nc.vector.tensor_copy(ident_b[:], ident[:])
            wt_const = None
            if mm_only:
                wt_const = const_pool.tile([P, NC_CHUNKS, P], BF16)
                nc.vector.memset(wt_const[:], 0.001)

            for b in [b for _ in range(reps) for b in range(B_PER_CORE)]:
                e_r = e_r_pool.tile([P, NC_CHUNKS, D], BF16, name="e_r")
                nc.gpsimd.dma_start(e_r[:], e_dram[b])
                etr = etr_pool.tile([P, ND_CHUNKS, T], F32R, name="etr")
                nc.gpsimd.dma_start(etr[:], et_dram[b])

                def emit_front(qb, b=b, etr=etr):
                    """DMA Q^T block qb, run 1-pass f32r bmm1. dc outer so
                    consecutive matmul pairs share the stationary operand.
                    Returns the scores PSUM tile."""
                    qt = qt_pool.tile([P, ND_CHUNKS, P], F32R, name="qt")
                    nc.sync.dma_start(qt[:], q_dram[b, qb])
                    sc_ps = sc_psum.tile([P, T], F32, name="sc_ps")
                    for dc in range(ND_CHUNKS):
                        for kh in range(2):
                            nc.tensor.matmul(
                                sc_ps[:, kh * 512 : (kh + 1) * 512],
                                qt[:, dc, :],
                                etr[:, dc, kh * 512 : (kh + 1) * 512],
                                start=(dc == 0),
                                stop=(dc == ND_CHUNKS - 1),
                            )
                    return sc_ps

                def emit_back(qb, sc_ps, b=b, e_r=e_r):
                    """Softmax block qb's scores, transpose W, bmm2, store."""
                    if mm_only:
                        # ablation: skip softmax + W transpose; bmm2 against a
                        # constant stationary operand. Output is garbage —
                        # timing only.
                        ctx_ps = ctx_psum.tile([P, T], F32, name="ctx_ps")
                        for kc in range(NC_CHUNKS):
                            for dh in range(2):
                                nc.tensor.matmul(
                                    ctx_ps[:, dh * 512 : (dh + 1) * 512],
                                    wt_const[:, kc, :],
                                    e_r[:, kc, dh * 512 : (dh + 1) * 512],
                                    start=(kc == 0),
                                    stop=(kc == NC_CHUNKS - 1),
                                )
                        ctx_sb = ctx_pool.tile([P, D], F32, name="ctx_sb")
                        nc.vector.tensor_copy(ctx_sb[:], ctx_ps[:])
                        # keep sc_ps live so bmm1 isn't dead code
                        nc.vector.tensor_copy(ctx_sb[:, 0:1], sc_ps[:, 0:1])
                        nc.sync.dma_start(
                            o_dram[b, qb * P : (qb + 1) * P, :], ctx_sb[:]
                        )
                        return
                    negmax = stat_pool.tile([P, 1], F32, tag="negmax", name="negmax")
                    nc.vector.tensor_reduce(
                        out=negmax[:],
                        in_=sc_ps[:],
                        op=mybir.AluOpType.max,
                        axis=mybir.AxisListType.X,
                        negate=True,
                    )
                    # exp (bf16 output) with fused row-sum accumulation
                    w_sb = w_pool.tile([P, T], BF16, name="w_sb")
                    ssum = stat_pool.tile([P, 1], F32, tag="ssum", name="ssum")
                    nc.scalar.activation(
                        w_sb[:],
                        sc_ps[:],
                        mybir.ActivationFunctionType.Exp,
                        bias=negmax[:],
                        accum_out=ssum[:],
                    )
                    recip = stat_pool.tile([P, 1], F32, tag="recip", name="recip")
                    nc.vector.reciprocal(recip[:], ssum[:])

                    # W^T via PE transpose-mode (bf16, 1 cyc/row), 4 blocks
                    # per PSUM bank; copies split ACT/DVE.
                    wt = wt_pool.tile([P, NC_CHUNKS, P], BF16, name="wt")
                    for g in range(2):
                        tp = trans_psum.tile([P, 2 * 4 * P], BF16, name="tp")
                        for j in range(4):
                            nc.tensor.transpose(
                                tp[:, j * P : (j + 1) * P],
                                w_sb[:, (g * 4 + j) * P : (g * 4 + j + 1) * P],
                                ident_b[:],
                            )
                        if g == 0:
                            nc.scalar.copy(wt[:, 0:4, :], tp[:, 0 : 4 * P])
                        else:
                            nc.vector.tensor_copy(wt[:, 4:8, :], tp[:, 0 : 4 * P])

                    # bmm2: ctx[q,d] = WT.T @ E. kc outer so matmuls start
                    # once the first W half's transposes land; dh inner
                    # alternates the two PSUM banks of one [P, 1024] tile.
                    ctx_ps = ctx_psum.tile([P, T], F32, name="ctx_ps")
                    for kc in range(NC_CHUNKS):
                        for dh in range(2):
                            nc.tensor.matmul(
                                ctx_ps[:, dh * 512 : (dh + 1) * 512],
                                wt[:, kc, :],
                                e_r[:, kc, dh * 512 : (dh + 1) * 512],
                                start=(kc == 0),
                                stop=(kc == NC_CHUNKS - 1),
                            )
                    ctx_sb = ctx_pool.tile([P, D], F32, name="ctx_sb")
                    nc.vector.tensor_scalar_mul(ctx_sb[:], ctx_ps[:], recip[:])
                    nc.sync.dma_start(o_dram[b, qb * P : (qb + 1) * P, :], ctx_sb[:])

                # software pipeline: next block's bmm1 hides this block's
                # softmax + W transpose + bmm2 tail latency.
                pend = emit_front(0)
                for qb in range(NC_CHUNKS):
                    nxt = emit_front(qb + 1) if qb + 1 < NC_CHUNKS else None
                    emit_back(qb, pend)
                    pend = nxt

    nc.compile()
    return nc


def make_in_maps(decoder_hidden: np.ndarray, encoder_outputs: np.ndarray):
    """Host-side sharding + layout prep: per-core input dicts matching the
    DRAM tensor layouts declared in build_nc."""
    dh = np.asarray(decoder_hidden, dtype=np.float32)
    eo = np.asarray(encoder_outputs, dtype=np.float32)
    assert dh.shape == (16, T, D) and eo.shape == (16, T, D)
    in_maps = []
    for i in range(8):
        qc = dh[i * B_PER_CORE : (i + 1) * B_PER_CORE]
        ec = eo[i * B_PER_CORE : (i + 1) * B_PER_CORE]
        # [b, qb, j, dc, p] -> [b, qb, p, dc, j]
        qh = np.ascontiguousarray(
            qc.reshape(B_PER_CORE, NC_CHUNKS, P, ND_CHUNKS, P).transpose(0, 1, 4, 3, 2)
        ).reshape(B_PER_CORE, NC_CHUNKS, P, D)
        # [b, kc, p, d] -> [b, p, kc, d], bf16 for bmm2's rhs
        import ml_dtypes

        eh = np.ascontiguousarray(
            ec.reshape(B_PER_CORE, NC_CHUNKS, P, D)
            .transpose(0, 2, 1, 3)
            .astype(ml_dtypes.bfloat16)
        ).reshape(B_PER_CORE, P, NC_CHUNKS * D)
        # [b, k, dc, p] -> [b, p, dc, k]
        eth = np.ascontiguousarray(
            ec.reshape(B_PER_CORE, T, ND_CHUNKS, P).transpose(0, 3, 2, 1)
        ).reshape(B_PER_CORE, P, ND_CHUNKS * T)
        in_maps.append({"q": qh, "e": eh, "et": eth})
    return in_maps


_NC_CACHE = None


def _get_nc():
    global _NC_CACHE
    if _NC_CACHE is None:
        _NC_CACHE = build_nc()
    return _NC_CACHE


def kernel(decoder_hidden: np.ndarray, encoder_outputs: np.ndarray) -> np.ndarray:
    import os

    # The axon client here has no NTFF profiling hook; make sure a stray
    # BASS_TRACE in the environment can't push run_bass_kernel_spmd onto
    # the tracing path.
    os.environ["BASS_NEVER_TRACE"] = "1"
    from concourse import bass_utils

    nc = _get_nc()
    in_maps = make_in_maps(decoder_hidden, encoder_outputs)
    res = bass_utils.run_bass_kernel_spmd(nc, in_maps, core_ids=list(range(8)))
    return np.concatenate([r["o"] for r in res.results], axis=0)


# revision 23
# speedup vs baseline: 1.1595x; 1.1595x over previous
"""Trainium2 Bass kernel for unscaled Luong dot-product attention.

Problem: B=16, Tq=Tk=D=1024, fp32.
    scores = Q @ E^T ; weights = softmax(scores, -1) ; out = weights @ E

Sharding: data-parallel over batch — each of the 8 NeuronCores processes
2 batches end-to-end; no cross-core communication.

Layout strategy: the host-side sharding step (inside kernel()) rearranges
each core's inputs so no on-device transposition of Q or E is needed:
  - q is shipped per q-block as [qb, d-part, dc, j] (i.e. Q^T tiled), so
    each 128-row q-block's stationary operands DMA straight into SBUF.
  - e is shipped twice: natural [k-part, kc, d] (bmm2 rhs) and transposed
    [d-part, dc, k] (bmm1 rhs). One 4 MB DMA each per batch.
q/et are declared float32r: the PE reads the raw fp32 bits at its
full-rate reduced internal precision (~15-16 effective mantissa bits).
The weights/bmm2 path runs in bf16 (softmax weights are smooth in
[0,1]; error there is not amplified by the softmax). Measured rel_l2 =
1.85e-3 vs the fp32 reference; the gate is 2e-2. Only the softmax
weights, which are produced on device, still go through a PE transpose
(bf16, 1 cyc/row).

Per-core pipeline per batch, per 128-row q-block (software-pipelined:
block qb+1's bmm1 overlaps block qb's softmax/bmm2 tail):
  front: DMA q-block tiles, bmm1 into PSUM kh-outer (the row-max of
    half 0 starts at the halfway point).
  back: negated row-max (DVE) -> exp with per-partition bias and fused
    row-sum (ACT, bf16 out) -> PE-transpose W (bf16) -> bmm2 kc-outer
    with paired stationary operands -> fold 1/rowsum into the
    PSUM->SBUF output copy (DVE) -> DMA out.

Measured (paired reps=32 differencing): ~146 us/core steady state, vs
324 us for the 3xTF32-split predecessor. An mm-only ablation times the
same, i.e. the matmul stream (incl. its serial per-MM weight loads) is
the wall; softmax/transposes are fully hidden.
"""

import numpy as np

import concourse.bass as bass
import concourse.tile as tile
from concourse import bacc, mybir
from concourse.masks import make_identity

P = 128
B_PER_CORE = 2
T = 1024  # Tq = Tk
D = 1024
NC_CHUNKS = T // P  # 8 k-chunks / q-blocks
ND_CHUNKS = D // P  # 8 d-chunks
F32 = mybir.dt.float32
F32R = mybir.dt.float32r
BF16 = mybir.dt.bfloat16
F16 = mybir.dt.float16


def build_nc(reps: int = 1, mm_only: bool = False, dma_wt: bool = False):
    nc = bacc.Bacc("TRN2", target_bir_lowering=False, debug=False)
    # q: [b, qb, p, dc*128+j] = Q[b, qb*128+j, dc*128+p]  (Q^T, block-tiled,
    # fp16: 11-bit mantissa keeps the softmax-amplified score error ~6e-3,
    # and non-fp32 stationaries get the 2x fast-weight-load path)
    q_dram = nc.dram_tensor(
        "q", [B_PER_CORE, NC_CHUNKS, P, D], F16, kind="ExternalInput"
    ).ap()
    # e: [b, p, kc*1024+d] = E[b, kc*128+p, d]  (natural, partition-tiled,
    # bf16 — bmm2's rhs; the softmax weights are bf16 anyway)
    e_dram = nc.dram_tensor(
        "e", [B_PER_CORE, P, NC_CHUNKS * D], BF16, kind="ExternalInput"
    ).ap()
    # et: [b, p, dc*1024+k] = E[b, k, dc*128+p]  (transposed, partition-tiled)
    et_dram = nc.dram_tensor(
        "et", [B_PER_CORE, P, ND_CHUNKS * T], F16, kind="ExternalInput"
    ).ap()
    o_dram = nc.dram_tensor("o", [B_PER_CORE, T, D], F32, kind="ExternalOutput").ap()

    with tile.TileContext(nc) as tc:
        with (
            tc.tile_pool(name="const", bufs=1) as const_pool,
            tc.tile_pool(name="e_r", bufs=2) as e_r_pool,
            tc.tile_pool(name="etr", bufs=2) as etr_pool,
            tc.tile_pool(name="qt", bufs=3) as qt_pool,
            tc.tile_pool(name="w", bufs=2) as w_pool,
            tc.tile_pool(name="wt", bufs=2) as wt_pool,
            tc.tile_pool(name="ctx", bufs=2) as ctx_pool,
            tc.tile_pool(name="stat", bufs=4) as stat_pool,
            tc.tile_pool(name="sc_ps", bufs=2, space="PSUM") as sc_psum,
            tc.tile_pool(name="ctx_ps", bufs=1, space="PSUM") as ctx_psum,
            tc.tile_pool(name="tr_ps", bufs=2, space="PSUM") as trans_psum,
        ):
            ident = const_pool.tile([P, P], F32)
            make_identity(nc, ident[:])
            ident_b = const_pool.tile([P, P], BF16)
            nc.vector.tensor_copy(ident_b[:], ident[:])
            wt_const = None
            if mm_only:
                wt_const = const_pool.tile([P, NC_CHUNKS, P], BF16)
                nc.vector.memset(wt_const[:], 0.001)

            for b in [b for _ in range(reps) for b in range(B_PER_CORE)]:
                e_r = e_r_pool.tile([P, NC_CHUNKS, D], BF16, name="e_r")
                nc.gpsimd.dma_start(e_r[:], e_dram[b])
                etr = etr_pool.tile([P, ND_CHUNKS, T], F16, name="etr")
                nc.gpsimd.dma_start(etr[:], et_dram[b])

                def emit_front(qb, b=b, etr=etr):
                    """DMA Q^T block qb, run 1-pass fp16 bmm1. dc outer so
                    consecutive matmul pairs share the stationary operand.
                    Returns the scores PSUM tile."""
                    qt = qt_pool.tile([P, ND_CHUNKS, P], F16, name="qt")
                    nc.sync.dma_start(qt[:], q_dram[b, qb])
                    sc_ps = sc_psum.tile([P, T], F32, name="sc_ps")
                    for dc in range(ND_CHUNKS):
                        for kh in range(2):
                            nc.tensor.matmul(
                                sc_ps[:, kh * 512 : (kh + 1) * 512],
                                qt[:, dc, :],
                                etr[:, dc, kh * 512 : (kh + 1) * 512],
                                start=(dc == 0),
                                stop=(dc == ND_CHUNKS - 1),
                            )
                    return sc_ps

                def emit_back(qb, sc_ps, b=b, e_r=e_r):
                    """Softmax block qb's scores, transpose W, bmm2, store."""
                    if mm_only:
                        # ablation: skip softmax + W transpose; bmm2 against a
                        # constant stationary operand. Output is garbage —
                        # timing only.
                        ctx_ps = ctx_psum.tile([P, T], F32, name="ctx_ps")
                        for kc in range(NC_CHUNKS):
                            for dh in range(2):
                                nc.tensor.matmul(
                                    ctx_ps[:, dh * 512 : (dh + 1) * 512],
                                    wt_const[:, kc, :],
                                    e_r[:, kc, dh * 512 : (dh + 1) * 512],
                                    start=(kc == 0),
                                    stop=(kc == NC_CHUNKS - 1),
                                )
                        ctx_sb = ctx_pool.tile([P, D], F32, name="ctx_sb")
                        nc.vector.tensor_copy(ctx_sb[:], ctx_ps[:])
                        # keep sc_ps live so bmm1 isn't dead code
                        nc.vector.tensor_copy(ctx_sb[:, 0:1], sc_ps[:, 0:1])
                        nc.sync.dma_start(
                            o_dram[b, qb * P : (qb + 1) * P, :], ctx_sb[:]
                        )
                        return
                    negmax = stat_pool.tile([P, 1], F32, tag="negmax", name="negmax")
                    nc.vector.tensor_reduce(
                        out=negmax[:],
                        in_=sc_ps[:],
                        op=mybir.AluOpType.max,
                        axis=mybir.AxisListType.X,
                        negate=True,
                    )
                    # exp (bf16 output) with fused row-sum accumulation
                    w_sb = w_pool.tile([P, T], BF16, name="w_sb")
                    ssum = stat_pool.tile([P, 1], F32, tag="ssum", name="ssum")
                    nc.scalar.activation(
                        w_sb[:],
                        sc_ps[:],
                        mybir.ActivationFunctionType.Exp,
                        bias=negmax[:],
                        accum_out=ssum[:],
                    )
                    recip = stat_pool.tile([P, 1], F32, tag="recip", name="recip")
                    nc.vector.reciprocal(recip[:], ssum[:])

                    wt = wt_pool.tile([P, NC_CHUNKS, P], BF16, name="wt")
                    if dma_wt:
                        # W^T via the DMA crossbar transpose (SBUF->SBUF,
                        # 2-byte dtype): frees the PE and the PSUM round trip.
                        nc.sync.dma_start_transpose(wt[:], w_sb[:])
                    else:
                        # W^T via PE transpose-mode (bf16, 1 cyc/row), 4
                        # blocks per PSUM bank; copies split ACT/DVE.
                        for g in range(2):
                            tp = trans_psum.tile([P, 2 * 4 * P], BF16, name="tp")
                            for j in range(4):
                                nc.tensor.transpose(
                                    tp[:, j * P : (j + 1) * P],
                                    w_sb[:, (g * 4 + j) * P : (g * 4 + j + 1) * P],
                                    ident_b[:],
                                )
                            if g == 0:
                                nc.scalar.copy(wt[:, 0:4, :], tp[:, 0 : 4 * P])
                            else:
                                nc.vector.tensor_copy(wt[:, 4:8, :], tp[:, 0 : 4 * P])

                    # bmm2: ctx[q,d] = WT.T @ E. kc outer so matmuls start
                    # once the first W half's transposes land; dh inner
                    # alternates the two PSUM banks of one [P, 1024] tile.
                    ctx_ps = ctx_psum.tile([P, T], F32, name="ctx_ps")
                    for kc in range(NC_CHUNKS):
                        for dh in range(2):
                            nc.tensor.matmul(
                                ctx_ps[:, dh * 512 : (dh + 1) * 512],
                                wt[:, kc, :],
                                e_r[:, kc, dh * 512 : (dh + 1) * 512],
                                start=(kc == 0),
                                stop=(kc == NC_CHUNKS - 1),
                            )
                    ctx_sb = ctx_pool.tile([P, D], F32, name="ctx_sb")
                    nc.vector.tensor_scalar_mul(ctx_sb[:], ctx_ps[:], recip[:])
                    nc.sync.dma_start(o_dram[b, qb * P : (qb + 1) * P, :], ctx_sb[:])

                # software pipeline: next block's bmm1 hides this block's
                # softmax + W transpose + bmm2 tail latency.
                pend = emit_front(0)
                for qb in range(NC_CHUNKS):
                    nxt = emit_front(qb + 1) if qb + 1 < NC_CHUNKS else None
                    emit_back(qb, pend)
                    pend = nxt

    nc.compile()
    return nc


def make_in_maps(decoder_hidden: np.ndarray, encoder_outputs: np.ndarray):
    """Host-side sharding + layout prep: per-core input dicts matching the
    DRAM tensor layouts declared in build_nc."""
    dh = np.asarray(decoder_hidden, dtype=np.float32)
    eo = np.asarray(encoder_outputs, dtype=np.float32)
    assert dh.shape == (16, T, D) and eo.shape == (16, T, D)
    in_maps = []
    for i in range(8):
        qc = dh[i * B_PER_CORE : (i + 1) * B_PER_CORE]
        ec = eo[i * B_PER_CORE : (i + 1) * B_PER_CORE]
        # [b, qb, j, dc, p] -> [b, qb, p, dc, j], fp16 (bmm1 stationary)
        qh = np.ascontiguousarray(
            qc.reshape(B_PER_CORE, NC_CHUNKS, P, ND_CHUNKS, P)
            .transpose(0, 1, 4, 3, 2)
            .astype(np.float16)
        ).reshape(B_PER_CORE, NC_CHUNKS, P, D)
        # [b, kc, p, d] -> [b, p, kc, d], bf16 for bmm2's rhs
        import ml_dtypes

        eh = np.ascontiguousarray(
            ec.reshape(B_PER_CORE, NC_CHUNKS, P, D)
            .transpose(0, 2, 1, 3)
            .astype(ml_dtypes.bfloat16)
        ).reshape(B_PER_CORE, P, NC_CHUNKS * D)
        # [b, k, dc, p] -> [b, p, dc, k], fp16 (bmm1 moving operand)
        eth = np.ascontiguousarray(
            ec.reshape(B_PER_CORE, T, ND_CHUNKS, P)
            .transpose(0, 3, 2, 1)
            .astype(np.float16)
        ).reshape(B_PER_CORE, P, ND_CHUNKS * T)
        in_maps.append({"q": qh, "e": eh, "et": eth})
    return in_maps


_NC_CACHE = None


def _get_nc():
    global _NC_CACHE
    if _NC_CACHE is None:
        _NC_CACHE = build_nc()
    return _NC_CACHE


def kernel(decoder_hidden: np.ndarray, encoder_outputs: np.ndarray) -> np.ndarray:
    import os

    # The axon client here has no NTFF profiling hook; make sure a stray
    # BASS_TRACE in the environment can't push run_bass_kernel_spmd onto
    # the tracing path.
    os.environ["BASS_NEVER_TRACE"] = "1"
    from concourse import bass_utils

    nc = _get_nc()
    in_maps = make_in_maps(decoder_hidden, encoder_outputs)
    res = bass_utils.run_bass_kernel_spmd(nc, in_maps, core_ids=list(range(8)))
    return np.concatenate([r["o"] for r in res.results], axis=0)


# revision 25
# speedup vs baseline: 1.2593x; 1.0861x over previous
"""Trainium2 Bass kernel for unscaled Luong dot-product attention.

Problem: B=16, Tq=Tk=D=1024, fp32.
    scores = Q @ E^T ; weights = softmax(scores, -1) ; out = weights @ E

Sharding: data-parallel over batch — each of the 8 NeuronCores processes
2 batches end-to-end; no cross-core communication.

Layout strategy: the host-side sharding step (inside kernel()) rearranges
each core's inputs so no on-device transposition of Q or E is needed:
  - q is shipped per q-block as [qb, d-part, dc, j] (i.e. Q^T tiled), so
    each 128-row q-block's stationary operands DMA straight into SBUF.
  - e is shipped twice: natural [k-part, kc, d] (bmm2 rhs) and transposed
    [d-part, dc, k] (bmm1 rhs). One 4 MB DMA each per batch.
q/et are shipped as fp16: 4-byte (fp32/f32r) stationary operands pay a
serially-exposed ~107ns weight load per matmul (no fast-weight-load
packing), while fp16 weights load 2x faster — and fp16's 11-bit
mantissa keeps the softmax-amplified score error small. The
weights/bmm2 path runs in bf16 (softmax weights are smooth in [0,1];
error there is not amplified). Measured rel_l2 = 2.284e-3 vs the fp32
reference; the gate is 2e-2. Only the softmax weights, which are
produced on device, still go through a PE transpose (bf16, 1 cyc/row
— a DMA-crossbar transpose variant measured slower: its ~1-2us fixed
latency lands on the exp->W^T->bmm2 critical chain).

Per-core pipeline per batch, per 128-row q-block (software-pipelined:
block qb+1's bmm1 overlaps block qb's softmax/bmm2 tail):
  front: DMA q-block tiles, bmm1 into PSUM kh-outer (the row-max of
    half 0 starts at the halfway point).
  back: negated row-max (DVE) -> exp with per-partition bias and fused
    row-sum (ACT, bf16 out) -> PE-transpose W (bf16) -> bmm2 kc-outer
    with paired stationary operands -> fold 1/rowsum into the
    PSUM->SBUF output copy (DVE) -> DMA out.

Measured (paired reps=32 differencing): ~131 us/core steady state, vs
324 us for the 3xTF32-split predecessor (146 us for its f32r-bmm1
revision). An mm-only ablation times the same as the full kernel, i.e.
the matmul stream (incl. its per-MM weight loads) is the wall;
softmax/transposes are fully hidden behind it.
"""

import numpy as np

import concourse.bass as bass
import concourse.tile as tile
from concourse import bacc, mybir
from concourse.masks import make_identity

P = 128
B_PER_CORE = 2
T = 1024  # Tq = Tk
D = 1024
NC_CHUNKS = T // P  # 8 k-chunks / q-blocks
ND_CHUNKS = D // P  # 8 d-chunks
F32 = mybir.dt.float32
F32R = mybir.dt.float32r
BF16 = mybir.dt.bfloat16
F16 = mybir.dt.float16


def build_nc(reps: int = 1, mm_only: bool = False, dma_wt: bool = False):
    nc = bacc.Bacc("TRN2", target_bir_lowering=False, debug=False)
    # q: [b, qb, p, dc*128+j] = Q[b, qb*128+j, dc*128+p]  (Q^T, block-tiled,
    # fp16: 11-bit mantissa keeps the softmax-amplified score error ~6e-3,
    # and non-fp32 stationaries get the 2x fast-weight-load path)
    q_dram = nc.dram_tensor(
        "q", [B_PER_CORE, NC_CHUNKS, P, D], F16, kind="ExternalInput"
    ).ap()
    # e: [b, p, kc*1024+d] = E[b, kc*128+p, d]  (natural, partition-tiled,
    # bf16 — bmm2's rhs; the softmax weights are bf16 anyway)
    e_dram = nc.dram_tensor(
        "e", [B_PER_CORE, P, NC_CHUNKS * D], BF16, kind="ExternalInput"
    ).ap()
    # et: [b, p, dc*1024+k] = E[b, k, dc*128+p]  (transposed, partition-tiled)
    et_dram = nc.dram_tensor(
        "et", [B_PER_CORE, P, ND_CHUNKS * T], F16, kind="ExternalInput"
    ).ap()
    o_dram = nc.dram_tensor("o", [B_PER_CORE, T, D], F32, kind="ExternalOutput").ap()

    with tile.TileContext(nc) as tc:
        with (
            tc.tile_pool(name="const", bufs=1) as const_pool,
            tc.tile_pool(name="e_r", bufs=2) as e_r_pool,
            tc.tile_pool(name="etr", bufs=2) as etr_pool,
            tc.tile_pool(name="qt", bufs=3) as qt_pool,
            tc.tile_pool(name="w", bufs=2) as w_pool,
            tc.tile_pool(name="wt", bufs=2) as wt_pool,
            tc.tile_pool(name="ctx", bufs=2) as ctx_pool,
            tc.tile_pool(name="stat", bufs=4) as stat_pool,
            tc.tile_pool(name="sc_ps", bufs=2, space="PSUM") as sc_psum,
            tc.tile_pool(name="ctx_ps", bufs=1, space="PSUM") as ctx_psum,
            tc.tile_pool(name="tr_ps", bufs=2, space="PSUM") as trans_psum,
        ):
            ident = const_pool.tile([P, P], F32)
            make_identity(nc, ident[:])
            ident_b = const_pool.tile([P, P], BF16)
            nc.vector.tensor_copy(ident_b[:], ident[:])
            wt_const = None
            if mm_only:
                wt_const = const_pool.tile([P, NC_CHUNKS, P], BF16)
                nc.vector.memset(wt_const[:], 0.001)

            for b in [b for _ in range(reps) for b in range(B_PER_CORE)]:
                e_r = e_r_pool.tile([P, NC_CHUNKS, D], BF16, name="e_r")
                nc.gpsimd.dma_start(e_r[:], e_dram[b])
                etr = etr_pool.tile([P, ND_CHUNKS, T], F16, name="etr")
                nc.gpsimd.dma_start(etr[:], et_dram[b])

                def emit_front(qb, b=b, etr=etr):
                    """DMA Q^T block qb, run 1-pass fp16 bmm1. dc outer so
                    consecutive matmul pairs share the stationary operand.
                    Returns the scores PSUM tile."""
                    qt = qt_pool.tile([P, ND_CHUNKS, P], F16, name="qt")
                    nc.sync.dma_start(qt[:], q_dram[b, qb])
                    sc_ps = sc_psum.tile([P, T], F32, name="sc_ps")
                    for dc in range(ND_CHUNKS):
                        for kh in range(2):
                            nc.tensor.matmul(
                                sc_ps[:, kh * 512 : (kh + 1) * 512],
                                qt[:, dc, :],
                                etr[:, dc, kh * 512 : (kh + 1) * 512],
                                start=(dc == 0),
                                stop=(dc == ND_CHUNKS - 1),
                            )
                    return sc_ps

                def emit_back(qb, sc_ps, b=b, e_r=e_r):
                    """Softmax block qb's scores, transpose W, bmm2, store."""
                    if mm_only:
                        # ablation: skip softmax + W transpose; bmm2 against a
                        # constant stationary operand. Output is garbage —
                        # timing only.
                        ctx_ps = ctx_psum.tile([P, T], F32, name="ctx_ps")
                        for kc in range(NC_CHUNKS):
                            for dh in range(2):
                                nc.tensor.matmul(
                                    ctx_ps[:, dh * 512 : (dh + 1) * 512],
                                    wt_const[:, kc, :],
                                    e_r[:, kc, dh * 512 : (dh + 1) * 512],
                                    start=(kc == 0),
                                    stop=(kc == NC_CHUNKS - 1),
                                )
                        ctx_sb = ctx_pool.tile([P, D], F32, name="ctx_sb")
                        nc.vector.tensor_copy(ctx_sb[:], ctx_ps[:])
                        # keep sc_ps live so bmm1 isn't dead code
                        nc.vector.tensor_copy(ctx_sb[:, 0:1], sc_ps[:, 0:1])
                        nc.sync.dma_start(
                            o_dram[b, qb * P : (qb + 1) * P, :], ctx_sb[:]
                        )
                        return
                    negmax = stat_pool.tile([P, 1], F32, tag="negmax", name="negmax")
                    nc.vector.tensor_reduce(
                        out=negmax[:],
                        in_=sc_ps[:],
                        op=mybir.AluOpType.max,
                        axis=mybir.AxisListType.X,
                        negate=True,
                    )
                    # exp (bf16 output) with fused row-sum accumulation
                    w_sb = w_pool.tile([P, T], BF16, name="w_sb")
                    ssum = stat_pool.tile([P, 1], F32, tag="ssum", name="ssum")
                    nc.scalar.activation(
                        w_sb[:],
                        sc_ps[:],
                        mybir.ActivationFunctionType.Exp,
                        bias=negmax[:],
                        accum_out=ssum[:],
                    )
                    recip = stat_pool.tile([P, 1], F32, tag="recip", name="recip")
                    nc.vector.reciprocal(recip[:], ssum[:])

                    wt = wt_pool.tile([P, NC_CHUNKS, P], BF16, name="wt")
                    if dma_wt:
                        # W^T via the DMA crossbar transpose (SBUF->SBUF,
                        # 2-byte dtype): frees the PE and the PSUM round trip.
                        nc.sync.dma_start_transpose(wt[:], w_sb[:])
                    else:
                        # W^T via PE transpose-mode (bf16, 1 cyc/row), 4
                        # blocks per PSUM bank; copies split ACT/DVE.
                        for g in range(2):
                            tp = trans_psum.tile([P, 2 * 4 * P], BF16, name="tp")
                            for j in range(4):
                                nc.tensor.transpose(
                                    tp[:, j * P : (j + 1) * P],
                                    w_sb[:, (g * 4 + j) * P : (g * 4 + j + 1) * P],
                                    ident_b[:],
                                )
                            if g == 0:
                                nc.scalar.copy(wt[:, 0:4, :], tp[:, 0 : 4 * P])
                            else:
                                nc.vector.tensor_copy(wt[:, 4:8, :], tp[:, 0 : 4 * P])

                    # bmm2: ctx[q,d] = WT.T @ E. kc outer so matmuls start
                    # once the first W half's transposes land; dh inner
                    # alternates the two PSUM banks of one [P, 1024] tile.
                    ctx_ps = ctx_psum.tile([P, T], F32, name="ctx_ps")
                    for kc in range(NC_CHUNKS):
                        for dh in range(2):
                            nc.tensor.matmul(
                                ctx_ps[:, dh * 512 : (dh + 1) * 512],
                                wt[:, kc, :],
                                e_r[:, kc, dh * 512 : (dh + 1) * 512],
                                start=(kc == 0),
                                stop=(kc == NC_CHUNKS - 1),
                            )
                    ctx_sb = ctx_pool.tile([P, D], F32, name="ctx_sb")
                    nc.vector.tensor_scalar_mul(ctx_sb[:], ctx_ps[:], recip[:])
                    nc.sync.dma_start(o_dram[b, qb * P : (qb + 1) * P, :], ctx_sb[:])

                # software pipeline: next block's bmm1 hides this block's
                # softmax + W transpose + bmm2 tail latency.
                pend = emit_front(0)
                for qb in range(NC_CHUNKS):
                    nxt = emit_front(qb + 1) if qb + 1 < NC_CHUNKS else None
                    emit_back(qb, pend)
                    pend = nxt

    nc.compile()
    return nc


def make_in_maps(decoder_hidden: np.ndarray, encoder_outputs: np.ndarray):
    """Host-side sharding + layout prep: per-core input dicts matching the
    DRAM tensor layouts declared in build_nc."""
    dh = np.asarray(decoder_hidden, dtype=np.float32)
    eo = np.asarray(encoder_outputs, dtype=np.float32)
    assert dh.shape == (16, T, D) and eo.shape == (16, T, D)
    in_maps = []
    for i in range(8):
        qc = dh[i * B_PER_CORE : (i + 1) * B_PER_CORE]
        ec = eo[i * B_PER_CORE : (i + 1) * B_PER_CORE]
        # [b, qb, j, dc, p] -> [b, qb, p, dc, j], fp16 (bmm1 stationary)
        qh = np.ascontiguousarray(
            qc.reshape(B_PER_CORE, NC_CHUNKS, P, ND_CHUNKS, P)
            .transpose(0, 1, 4, 3, 2)
            .astype(np.float16)
        ).reshape(B_PER_CORE, NC_CHUNKS, P, D)
        # [b, kc, p, d] -> [b, p, kc, d], bf16 for bmm2's rhs
        import ml_dtypes

        eh = np.ascontiguousarray(
            ec.reshape(B_PER_CORE, NC_CHUNKS, P, D)
            .transpose(0, 2, 1, 3)
            .astype(ml_dtypes.bfloat16)
        ).reshape(B_PER_CORE, P, NC_CHUNKS * D)
        # [b, k, dc, p] -> [b, p, dc, k], fp16 (bmm1 moving operand)
        eth = np.ascontiguousarray(
            ec.reshape(B_PER_CORE, T, ND_CHUNKS, P)
            .transpose(0, 3, 2, 1)
            .astype(np.float16)
        ).reshape(B_PER_CORE, P, ND_CHUNKS * T)
        in_maps.append({"q": qh, "e": eh, "et": eth})
    return in_maps


_NC_CACHE = None


def _get_nc():
    global _NC_CACHE
    if _NC_CACHE is None:
        _NC_CACHE = build_nc()
    return _NC_CACHE


def kernel(decoder_hidden: np.ndarray, encoder_outputs: np.ndarray) -> np.ndarray:
    import os

    # The axon client here has no NTFF profiling hook; make sure a stray
    # BASS_TRACE in the environment can't push run_bass_kernel_spmd onto
    # the tracing path.
    os.environ["BASS_NEVER_TRACE"] = "1"
    from concourse import bass_utils

    nc = _get_nc()
    in_maps = make_in_maps(decoder_hidden, encoder_outputs)
    res = bass_utils.run_bass_kernel_spmd(nc, in_maps, core_ids=list(range(8)))
    return np.concatenate([r["o"] for r in res.results], axis=0)
